# revision 39
# baseline (speedup 1.0000x reference)
"""Trainium2 Bass kernel for nn_MultiMarginRankingLoss (128 queries x 1024 preds).

loss = -0.5 * rank_term + 0.5 * bce
  rank_term = mean_q [ sum_{p nonrel, j rel} relu(pred[q,p]-pred[q,j]) / 1024 ]
  bce       = mean( softplus(pred) - pred*yf )

Strategy: data-parallel over queries (16 per core, 8 cores). Per core, the
O(p^2) rank term is computed in O(p log^2 p) via a bitonic sort:
sort each query's 1024 values with the relevance flag packed into the fp32
mantissa LSB, then prefix-scan (HW scan op) of relevant counts/sums gives
  S_q = sum_{p nonrel} (v_p * cntRel(<p) - sumRel(<p)).

Layout: per-core tile X[128, 128]: partition pi = c*16 + q (c = chunk of 128
positions, q = query), free f = position within chunk. Sort position
i = c*128 + f. Compare-exchange stages along f are lane-local DVE min/max ops;
cross-chunk stages run in a transposed layout (TensorE transpose) so they are
also lane-local. All-ascending "normalized" bitonic: the first stage of each
merge level reads the second half block reversed (negative-stride AP).

The scalar loss terms are reduced on-chip to per-core partials [1,4]; the
host sums 8 tiny vectors and forms the final scalar.
"""
import sys

for _p in ("/opt/trn_rl_repo", "/root/.axon_site/_ro/trn_rl_repo"):
    if _p not in sys.path:
        sys.path.append(_p)

import numpy as np
from contextlib import ExitStack

import concourse.bass as bass
import concourse.bacc as bacc
import concourse.tile as tile
from concourse import mybir
from concourse.bass_utils import run_bass_kernel_spmd

F32 = mybir.dt.float32
I32 = mybir.dt.int32
OP = mybir.AluOpType

N_CORES = 8
NUM_Q, NUM_P = 128, 1024
QPC = NUM_Q // N_CORES  # 16 queries per core


def _pap(t, p0, pc, free_dims, off=0):
    """Partition-restricted AP over tile t with custom free dims."""
    base = t[p0:p0 + pc, :]
    return bass.AP(tensor=base.tensor, offset=base.offset + off,
                   ap=[base.ap[0]] + [list(d) for d in free_dims])


def _fap(t, free_dims, off=0):
    """AP over tile t with custom free dims (element units), all 128 partitions."""
    base = t[:]
    return bass.AP(tensor=base.tensor, offset=base.offset + off,
                   ap=[base.ap[0]] + [list(d) for d in free_dims])


def _emit_minmax(nc, lo, hi, a, b, allow_gp=True):
    maxeng = nc.gpsimd if (GPSPLIT and allow_gp) else nc.vector
    nc.vector.tensor_tensor(out=lo, in0=a, in1=b, op=OP.min)
    maxeng.tensor_tensor(out=hi, in0=a, in1=b, op=OP.max)


def _emit_f_stage(nc, cur, nxt, j, first_k=None):
    """One f-internal CE stage. If first_k is set (= k of the merge level),
    emit the fused-reversal first stage for 2k blocks; else the standard
    ascending stage at distance j."""
    if first_k is not None:
        k = first_k
        nb = 128 // (2 * k)
        a = _fap(cur, [[2 * k, nb], [1, k]], 0)
        brev = _fap(cur, [[2 * k, nb], [-1, k]], 2 * k - 1)
        lo = _fap(nxt, [[2 * k, nb], [1, k]], 0)
        hi = _fap(nxt, [[2 * k, nb], [1, k]], k)
        _emit_minmax(nc, lo, hi, a, brev)
    else:
        nbj = 128 // (2 * j)
        a = _fap(cur, [[2 * j, nbj], [1, j]], 0)
        b = _fap(cur, [[2 * j, nbj], [1, j]], j)
        lo = _fap(nxt, [[2 * j, nbj], [1, j]], 0)
        hi = _fap(nxt, [[2 * j, nbj], [1, j]], j)
        _emit_minmax(nc, lo, hi, a, b)


KPHASE = int(os.environ.get("KPHASE", "99"))
GPSPLIT = int(os.environ.get("GPSPLIT", "0"))


def build_program():
    nc = bacc.Bacc("TRN2", target_bir_lowering=False, debug=False,
                   num_devices=N_CORES)
    data_d = nc.dram_tensor("data", [128, 256], F32, kind="ExternalInput")
    mats_d = nc.dram_tensor("mats", [128, 256], F32, kind="ExternalInput")
    out_d = nc.dram_tensor("out", [128, 4], F32, kind="ExternalOutput")

    with tile.TileContext(nc) as tc, ExitStack() as ctx:
        pool = ctx.enter_context(tc.tile_pool(name="sb", bufs=1))
        psum = ctx.enter_context(tc.tile_pool(name="ps", bufs=1, space="PSUM"))

        # ---- loads (packed: [pred | y-bits], [lmat | imat]) ----
        data = pool.tile([128, 256], F32, tag="data")
        mats = pool.tile([128, 256], F32, tag="mats")
        nc.sync.dma_start(data[:], data_d.ap())
        nc.scalar.dma_start(mats[:], mats_d.ap())
        X = data[:, 0:128]
        Yi = data[:, 128:256].bitcast(I32)
        lmat = mats[:, 0:128]
        imat = mats[:, 128:256]

        # ---- BCE on scalar+gpsimd engines (independent of the sort) ----
        # softplus(x) = relu(x) + ln(1 + exp(-|x|)); accumulate both row-wise
        sp_junk = pool.tile([128, 128], F32, tag="spj")
        sp_abs = pool.tile([128, 128], F32, tag="spa")
        relu_rows = pool.tile([128, 1], F32, tag="rlr")
        ln_rows = pool.tile([128, 1], F32, tag="lnr")
        AF = mybir.ActivationFunctionType
        nc.scalar.activation(sp_abs[:], X, AF.Abs)
        nc.scalar.activation(sp_abs[:], sp_abs[:], AF.Exp, scale=-1.0)
        nc.scalar.activation(sp_junk[:], sp_abs[:], AF.Ln, bias=1.0,
                             accum_out=ln_rows[:])
        nc.scalar.activation(sp_junk[:], X, AF.Relu, accum_out=relu_rows[:])
        sp_rows = pool.tile([128, 1], F32, tag="spr")
        nc.vector.tensor_tensor(out=sp_rows[:], in0=relu_rows[:],
                                in1=ln_rows[:], op=OP.add)
        yfF = pool.tile([128, 128], F32, tag="yfF")
        nc.gpsimd.tensor_copy(out=yfF[:], in_=Yi)
        xy_junk = pool.tile([128, 128], F32, tag="xyj")
        xy_rows = pool.tile([128, 1], F32, tag="xyr")
        nc.vector.scalar_tensor_tensor(out=xy_junk[:], in0=X, scalar=1.0,
                                       in1=yfF[:], op0=OP.mult, op1=OP.mult,
                                       accum_out=xy_rows[:])

        # ---- key pack: key = (pred_bits & ~1) | y  (sorts like pred, LSB=rel) ----
        sA = pool.tile([128, 128], F32, tag="sA")
        sB = pool.tile([128, 128], F32, tag="sB")
        tmpi = pool.tile([128, 128], I32, tag="tmpi")
        nc.vector.tensor_scalar(out=tmpi[:], in0=X.bitcast(I32), scalar1=-2,
                                scalar2=None, op0=OP.bitwise_and)
        nc.vector.tensor_tensor(out=sA[:].bitcast(I32), in0=tmpi[:], in1=Yi,
                                op=OP.bitwise_or)

        # ---- sort: f-internal levels (sorted 2k-blocks up to 128) ----
        cur, nxt = sA, sB
        fks = [1, 2, 4, 8, 16, 32, 64] if KPHASE >= 2 else []
        for k in fks:
            _emit_f_stage(nc, cur, nxt, None, first_k=k)
            cur, nxt = nxt, cur
            j = k // 2
            while j >= 1:
                _emit_f_stage(nc, cur, nxt, j)
                cur, nxt = nxt, cur
                j //= 2

        # ---- cross-chunk levels in transposed layout ----
        xr = pool.tile([128, 128], F32, tag="xr")
        tf = pool.tile([128, 128], F32, tag="tf")
        tr = pool.tile([128, 128], F32, tag="tr")
        tn = pool.tile([128, 128], F32, tag="tn")
        pt1 = psum.tile([128, 128], F32, tag="pt1")
        pt2 = psum.tile([128, 128], F32, tag="pt2")

        levels = {2: (), 3: (256,), 4: (256, 512), 45: (256, 512, 1024)}.get(KPHASE, (256, 512, 1024))
        if KPHASE < 2:
            levels = ()
        for level in levels:
            # f-reversed copy, then two PE transposes (fwd + reversed)
            nc.vector.tensor_copy(out=xr[:],
                                  in_=_fap(cur, [[-1, 128]], 127))
            nc.tensor.transpose(pt1[:], cur[:], imat)
            nc.tensor.transpose(pt2[:], xr[:], imat)
            # copy only the halves the entry stage reads: a-positions from
            # pt1 (ACT), b-positions from pt2 (DVE)
            if level == 256:
                apat, bpat = ([[32, 4], [1, 16]], 0), ([[32, 4], [1, 16]], 16)
            elif level == 512:
                apat, bpat = ([[64, 2], [16, 2], [1, 16]], 0), ([[64, 2], [16, 2], [1, 16]], 32)
            else:
                apat, bpat = ([[16, 4], [1, 16]], 0), ([[16, 4], [1, 16]], 64)
            nc.scalar.copy(out=_fap(tf, *apat), in_=_fap(pt1, *apat))
            nc.vector.tensor_copy(out=_fap(tr, *bpat), in_=_fap(pt2, *bpat))

            # entry stage: lo/hi with chunk-reversed second operand
            if level == 256:
                a = _fap(tf, [[32, 4], [1, 16]], 0)
                br = _fap(tr, [[32, 4], [1, 16]], 16)
                lo = _fap(tn, [[32, 4], [1, 16]], 0)
                hi = _fap(tn, [[32, 4], [1, 16]], 16)
            elif level == 512:
                a = _fap(tf, [[64, 2], [16, 2], [1, 16]], 0)
                br = _fap(tr, [[64, 2], [-16, 2], [1, 16]], 48)
                lo = _fap(tn, [[64, 2], [16, 2], [1, 16]], 0)
                hi = _fap(tn, [[64, 2], [16, 2], [1, 16]], 32)
            else:
                a = _fap(tf, [[16, 4], [1, 16]], 0)
                br = _fap(tr, [[-16, 4], [1, 16]], 112)
                lo = _fap(tn, [[16, 4], [1, 16]], 0)
                hi = _fap(tn, [[16, 4], [1, 16]], 64)
            _emit_minmax(nc, lo, hi, a, br, allow_gp=False)
            tcur, tnxt = tn, tf  # tf consumed; reuse as ping-pong partner

            # remaining cross stages (standard ascending, chunk distance d)
            dists = [] if level == 256 else ([1] if level == 512 else [2, 1])
            for d in dists:
                nb = 8 // (2 * d)
                if d == 1:
                    dims = [[32, nb], [1, 16]]
                    a = _fap(tcur, dims, 0); b = _fap(tcur, dims, 16)
                    lo = _fap(tnxt, dims, 0); hi = _fap(tnxt, dims, 16)
                else:
                    dims = [[2 * d * 16, nb], [16, d], [1, 16]]
                    a = _fap(tcur, dims, 0); b = _fap(tcur, dims, d * 16)
                    lo = _fap(tnxt, dims, 0); hi = _fap(tnxt, dims, d * 16)
                _emit_minmax(nc, lo, hi, a, b)
                tcur, tnxt = tnxt, tcur

            # transpose back, then f-internal merge stages j=64..1
            nc.tensor.transpose(pt1[:], tcur[:], imat)
            nc.scalar.copy(out=cur[:, 0:64], in_=pt1[:, 0:64])
            nc.vector.tensor_copy(out=cur[:, 64:128], in_=pt1[:, 64:128])
            j = 64
            while j >= 1:
                _emit_f_stage(nc, cur, nxt, j)
                cur, nxt = nxt, cur
                j //= 2

        S = cur  # sorted keys, ascending per query in (c-major, f) position order
        DO_EPI = KPHASE in (5, 99) or 51 <= KPHASE <= 56
        EP = KPHASE - 50 if 51 <= KPHASE <= 56 else 6

        # ---- epilogue: rank term via prefix scans ----
        flgI = pool.tile([128, 128], I32, tag="flgI")
        flgF = pool.tile([128, 128], F32, tag="flgF")
        valI = pool.tile([128, 128], I32, tag="valI")
        nc.vector.tensor_scalar(out=flgI[:], in0=S[:].bitcast(I32), scalar1=1,
                                scalar2=None, op0=OP.bitwise_and)
        nc.vector.tensor_copy(out=flgF[:], in_=flgI[:])
        nc.vector.tensor_scalar(out=valI[:], in0=S[:].bitcast(I32), scalar1=-2,
                                scalar2=None, op0=OP.bitwise_and)
        valF = valI[:].bitcast(F32)
        rv = pool.tile([128, 128], F32, tag="rv")
        nc.vector.tensor_tensor(out=rv[:], in0=valF, in1=flgF[:], op=OP.mult)

        z = pool.tile([128, 128], F32, tag="z")
        nc.gpsimd.memset(z[:], 0.0)
        Rinc = pool.tile([128, 128], F32, tag="Rinc")
        Tinc = pool.tile([128, 128], F32, tag="Tinc")
        nc.vector.tensor_tensor_scan(out=Rinc[:], data0=z[:], data1=flgF[:],
                                     initial=0.0, op0=OP.add, op1=OP.add)
        nc.vector.tensor_tensor_scan(out=Tinc[:], data0=z[:], data1=rv[:],
                                     initial=0.0, op0=OP.add, op1=OP.add)

        tots = pool.tile([128, 2], F32, tag="tots")
        nc.vector.tensor_copy(out=tots[:, 0:1], in_=Rinc[:, 127:128])
        nc.vector.tensor_copy(out=tots[:, 1:2], in_=Tinc[:, 127:128])
        pbase = psum.tile([128, 2], F32, tag="pbase")
        nc.tensor.matmul(pbase[:], lmat, tots[:])

        Rf = pool.tile([128, 128], F32, tag="Rf")
        Tf2 = pool.tile([128, 128], F32, tag="Tf2")
        nc.vector.scalar_tensor_tensor(out=Rf[:], in0=Rinc[:],
                                       scalar=pbase[:, 0:1], in1=flgF[:],
                                       op0=OP.add, op1=OP.subtract)
        nc.vector.scalar_tensor_tensor(out=Tf2[:], in0=Tinc[:],
                                       scalar=pbase[:, 1:2], in1=rv[:],
                                       op0=OP.add, op1=OP.subtract)
        m2 = pool.tile([128, 128], F32, tag="m2")
        nc.vector.tensor_tensor(out=m2[:], in0=valF, in1=Rf[:], op=OP.mult)
        nc.vector.tensor_tensor(out=m2[:], in0=m2[:], in1=Tf2[:], op=OP.subtract)

        junk = pool.tile([128, 128], F32, tag="junk")
        acc1 = pool.tile([128, 1], F32, tag="acc1")
        acc2 = pool.tile([128, 1], F32, tag="acc2")
        nc.vector.tensor_tensor_reduce(out=junk[:], in0=m2[:], in1=flgF[:],
                                       scale=1.0, scalar=0.0, op0=OP.mult,
                                       op1=OP.add, accum_out=acc1[:])
        nc.vector.tensor_reduce(out=acc2[:], in_=m2[:],
                                axis=mybir.AxisListType.X, op=OP.add)

        # ---- final per-core partials -> [1,4] ----
        stats = pool.tile([128, 4], F32, tag="stats")
        nc.vector.memset(stats[:], 0.0)
        nc.vector.tensor_tensor(out=stats[:, 0:1], in0=acc2[:], in1=acc1[:],
                                op=OP.subtract)
        nc.vector.tensor_copy(out=stats[:, 1:2], in_=sp_rows[:])
        nc.vector.tensor_copy(out=stats[:, 2:3], in_=xy_rows[:])
        nc.sync.dma_start(out_d.ap(), stats[:])

    nc.compile()
    return nc


_NC_CACHE = []


def _get_nc():
    if not _NC_CACHE:
        _NC_CACHE.append(build_program())
    return _NC_CACHE[0]


def _consts():
    pi = np.arange(128)
    lmat = ((pi[:, None] % 16 == pi[None, :] % 16)
            & (pi[:, None] // 16 < pi[None, :] // 16)).astype(np.float32)
    imat = np.eye(128, dtype=np.float32)
    return lmat, imat


def kernel(pred: np.ndarray, y: np.ndarray) -> np.ndarray:
    pred = np.ascontiguousarray(np.asarray(pred, dtype=np.float32))
    y = np.ascontiguousarray(np.asarray(y, dtype=np.int32))
    assert pred.shape == (NUM_Q, NUM_P) and y.shape == (NUM_Q, NUM_P)
    nc = _get_nc()
    lmat, imat = _consts()
    def chunked(a):
        return np.ascontiguousarray(
            a.reshape(QPC, 8, 128).transpose(1, 0, 2).reshape(128, 128))

    mats = np.concatenate([lmat, imat], axis=1)
    in_maps = []
    for c in range(N_CORES):
        sl = slice(c * QPC, (c + 1) * QPC)
        data = np.concatenate(
            [chunked(pred[sl]), chunked(y[sl]).view(np.float32)], axis=1)
        in_maps.append({"data": data, "mats": mats})
    r = run_bass_kernel_spmd(nc, in_maps, core_ids=list(range(N_CORES)))
    tot = np.zeros(3, dtype=np.float64)
    for c in range(N_CORES):
        tot += r.results[c]["out"][:, :3].astype(np.float64).sum(axis=0)
    rank_sum, sp_sum, xy_sum = tot
    rank_term = rank_sum / NUM_P / NUM_Q
    bce = (sp_sum - xy_sum) / (NUM_Q * NUM_P)
    loss = -0.5 * rank_term + 0.5 * bce
    return np.float32(loss)


if __name__ == "__main__":
    rng = np.random.default_rng(0)
    pred = rng.normal(size=(NUM_Q, NUM_P)).astype(np.float32)
    y = rng.integers(0, 2, size=(NUM_Q, NUM_P)).astype(np.int32)
    print("loss:", kernel(pred, y))
